# revision 12
# baseline (speedup 1.0000x reference)
"""Trainium2 Bass kernel for policy-masked attention (nn_Attention_5007931867377).

Reference computation (per batch b):
    qkv = x @ w_qkv.T ; split into q,k,v heads [H=6, N=1568, D=64]
    s   = (q @ k.T) * D**-0.5
    mask[m] visibility per key + diagonal always kept
    e   = exp(s - max) * mask ; attn = (e + EPS/N)/(sum e + EPS)
    out = (attn @ v) concat heads @ w_proj.T + b_proj

Strategy: pure data parallel, one batch element per NeuronCore (8 cores).
Per-core dataflow (everything transposed so softmax reductions are on the
free axis and e^T feeds the V-matmul without on-chip transposes):
  - host passes x^T; on chip: Q^T,K^T in [c_out, n] layout, V in [n, d].
  - scores^T[m, q] = K^T.T @ Q^T  (PSUM, per 128-key chunk x 512-query piece)
  - key-mask folded into exp as per-partition bias (-30 * (1-p));
    diagonal-keep folded into scores via +240*(1-p) diag matmul (exp scale
    0.125 makes that +30, cancelling the -30 bias exactly on the diagonal)
  - e^T = exp(0.125*s + bias) ACT -> SBUF
  - V augmented with a ones column: outT_aug[65, q] = V_aug.T @ e^T gives
    attention output rows 0..63 and the softmax denominator in row 64
  - normalize with reciprocal_approx_fast + a rank-1 broadcast matmul
  - proj: y[n, :] = oT.T @ w_proj.T (+ bias via K=1 ones matmul), DMA out.

Max-subtraction is dropped: scores ~ N(0,1), max |s| < ~6, exp stays well
inside f32 range and the EPS terms shift results by ~1e-10 relative.
"""

import sys

if "/opt/trn_rl_repo" not in sys.path:
    sys.path.insert(0, "/opt/trn_rl_repo")

import numpy as np

B, N, C, H = 8, 1568, 384, 6
D = C // H  # 64
SCALE = D ** -0.5  # 0.125
EPS = 1e-6
NEG = -30.0  # masked-key exp bias (exp(-30) ~ 9e-14, way below EPS/N)
DIAGV = -NEG / SCALE  # 240.0 added to diagonal scores, cancels bias exactly

P = 128
NCH = (N + P - 1) // P  # 13 key/token chunks (12 x 128 + 1 x 32)
CHS = [min(P, N - i * P) for i in range(NCH)]  # chunk sizes
QP = 512
QPIECES = [(o, min(QP, N - o)) for o in range(0, N, QP)]  # (offset, width)

_CACHE = {}


def _build_nc():
    import concourse.tile as tile
    from concourse import bacc, mybir

    dt = mybir.dt
    f32 = dt.float32
    f32r = dt.float32r
    bf16 = dt.bfloat16
    AF = mybir.ActivationFunctionType

    nc = bacc.Bacc()

    xT_d = nc.declare_dram_parameter("xT", [C, N], f32r, isOutput=False)
    wqkvT_d = nc.declare_dram_parameter("wqkvT", [C, 3 * C], f32r, isOutput=False)
    wprojT_d = nc.declare_dram_parameter("wprojT", [C, C], f32r, isOutput=False)
    bias_d = nc.declare_dram_parameter("bias_exp", [P, NCH], f32, isOutput=False)
    dfix_d = nc.declare_dram_parameter("dfix", [P, NCH, P], bf16, isOutput=False)
    ident_d = nc.declare_dram_parameter("ident", [P, P], bf16, isOutput=False)
    bvec_d = nc.declare_dram_parameter("bvec", [1, C], f32r, isOutput=False)
    out_d = nc.declare_dram_parameter("out", [N, C], f32, isOutput=True)

    def r(ap):  # matmul-feeding tensors are already float32r
        return ap

    with tile.TileContext(nc) as tc:
        with (
            tc.tile_pool(name="persist", bufs=1) as pp,
            tc.tile_pool(name="work", bufs=4) as wp,
        ):
            # ---- persistent SBUF tensors (float32r where they feed matmuls) ----
            xt = pp.tile([P, 3, N], f32r, tag="xt")  # x^T chunks (c rows)
            wqkv = pp.tile([P, 3, 3 * C], f32r, tag="wqkv")
            wproj = pp.tile([P, 3, C], f32r, tag="wproj")
            qk = pp.tile([P, 6, N], f32r, tag="qk")  # Q^T (0..2), K^T (3..5)
            vaug = pp.tile([P, NCH, H, D + 1], f32r, tag="vaug")
            ot = pp.tile([P, 3, N], f32r, tag="ot")  # normalized attn out ^T
            bias = pp.tile([P, NCH], f32, tag="bias")
            dfix = pp.tile([P, NCH, P], bf16, tag="dfix")
            ident = pp.tile([P, P], bf16, tag="ident")
            bvec = pp.tile([1, C], f32r, tag="bvec")
            ones = pp.tile([1, P], f32r, tag="ones")

            nc.sync.dma_start(xt[:, :, :], xT_d[:].rearrange("(a p) n -> p a n", p=P))
            nc.sync.dma_start(
                wqkv[:, :, :], wqkvT_d[:].rearrange("(a p) n -> p a n", p=P)
            )
            nc.sync.dma_start(
                wproj[:, :, :], wprojT_d[:].rearrange("(a p) n -> p a n", p=P)
            )
            nc.sync.dma_start(bias[:, :], bias_d[:])
            nc.sync.dma_start(dfix[:, :, :], dfix_d[:])
            nc.sync.dma_start(ident[:, :], ident_d[:])
            nc.sync.dma_start(bvec[:, :], bvec_d[:])
            nc.vector.memset(ones[:, :].bitcast(f32), 1.0)

            # ---- phase 1: qkv projections ----
            with tc.tile_pool(name="qkvps", bufs=3, space="PSUM") as qps:
                # Q^T / K^T: out[c_out 128, n] = wqkvT_chunk.T @ xT
                for cc in range(6):  # 6 chunks of 128 output channels (q, k)
                    for (qo, qw) in QPIECES:
                        ps = qps.tile([P, QP], f32, tag="qk")
                        for c in range(3):
                            nc.tensor.matmul(
                                ps[:, :qw],
                                r(wqkv[:, c, cc * P : (cc + 1) * P]),
                                r(xt[:, c, qo : qo + qw]),
                                start=(c == 0),
                                stop=(c == 2),
                            )
                        nc.vector.tensor_copy(qk[:, cc, qo : qo + qw], ps[:, :qw])
                # V: out[n 128, 384] = xT_chunk.T @ w_vT ; pack into vaug
                for i in range(NCH):
                    m = CHS[i]
                    ps = qps.tile([P, C], f32, tag="v")
                    for c in range(3):
                        nc.tensor.matmul(
                            ps[:m, :],
                            r(xt[:, c, i * P : i * P + m]),
                            r(wqkv[:, c, 2 * C : 3 * C]),
                            start=(c == 0),
                            stop=(c == 2),
                        )
                    nc.vector.tensor_copy(
                        vaug[:m, i, :, 0:D],
                        ps[:m, :].rearrange("p (h d) -> p h d", h=H),
                    )
                    nc.vector.memset(vaug[:m, i, :, D : D + 1].bitcast(f32), 1.0)

            # ---- phase 2: attention per head ----
            with (
                tc.tile_pool(name="outps", bufs=1, space="PSUM") as ops,
                tc.tile_pool(name="scps", bufs=4, space="PSUM") as sps,
            ):
                for h in range(H):
                    qc, qr = h // 2, (h % 2) * D  # Q^T chunk / row of head h
                    kc, kr = 3 + h // 2, (h % 2) * D
                    outp = ops.tile([D + 1, N], f32, tag="outT")
                    for i in range(NCH):
                        m = CHS[i]
                        for pi, (qo, qw) in enumerate(QPIECES):
                            sc = sps.tile([P, QP], f32, tag="sc")
                            has_diag = (i * P) // QP == pi
                            nc.tensor.matmul(
                                sc[:m, :qw],
                                r(qk[kr : kr + D, kc, i * P : i * P + m]),
                                r(qk[qr : qr + D, qc, qo : qo + qw]),
                                start=True,
                                stop=not has_diag,
                            )
                            if has_diag:
                                do = i * P - qo
                                nc.tensor.matmul(
                                    sc[:m, do : do + m],
                                    ident[:, :m],
                                    dfix[:, i, :m],
                                    start=False,
                                    stop=True,
                                )
                            et = wp.tile([P, QP], f32r, tag="et")
                            nc.scalar.activation(
                                et[:m, :qw],
                                sc[:m, :qw],
                                AF.Exp,
                                bias=bias[:m, i : i + 1],
                                scale=SCALE,
                            )
                            nc.tensor.matmul(
                                outp[:, qo : qo + qw],
                                r(vaug[:m, i, h, :]),
                                r(et[:m, :qw]),
                                start=(i == 0),
                                stop=(i == NCH - 1),
                            )
                    # normalization: S = row 64 of outp
                    srow = wp.tile([1, N], f32, tag="srow")
                    srow_r = wp.tile([1, N], f32r, tag="srow_r")
                    nc.vector.tensor_scalar_add(srow[:, :], outp[D : D + 1, :], EPS)
                    nc.vector.reciprocal_approx_fast(srow[:, :], srow[:, :])
                    nc.vector.tensor_copy(srow_r[:, :], srow[:, :])
                    for (qo, qw) in QPIECES:
                        rb = sps.tile([P, QP], f32, tag="sc")  # reuse bank slots
                        nc.tensor.matmul(
                            rb[:D, :qw],
                            r(ones[:, :D]),
                            r(srow_r[:, qo : qo + qw]),
                        )
                        rbs = wp.tile([D, QP], f32, tag="rbs")
                        nc.vector.tensor_copy(rbs[:, :qw], rb[:D, :qw])
                        nc.vector.tensor_mul(
                            ot[qr : qr + D, qc, qo : qo + qw],
                            outp[0:D, qo : qo + qw],
                            rbs[:, :qw],
                        )

            # ---- phase 3: output projection ----
            with tc.tile_pool(name="yps", bufs=3, space="PSUM") as yps:
                for j in range(NCH):
                    m = CHS[j]
                    yp = yps.tile([P, C], f32, tag="y")
                    for c in range(3):
                        nc.tensor.matmul(
                            yp[:m, :],
                            r(ot[:, c, j * P : j * P + m]),
                            r(wproj[:, c, :]),
                            start=(c == 0),
                            stop=False,
                        )
                    nc.tensor.matmul(
                        yp[:m, :],
                        r(ones[:, :m]),
                        r(bvec[:, :]),
                        start=False,
                        stop=True,
                    )
                    ys = wp.tile([P, C], f32, tag="ys")
                    nc.vector.tensor_copy(ys[:m, :], yp[:m, :])
                    nc.sync.dma_start(out_d[j * P : j * P + m, :], ys[:m, :])

    nc.finalize()
    return nc


def _prep_core_inputs(x_b, p_b, w_qkv, w_proj, b_proj):
    import ml_dtypes

    bf16 = ml_dtypes.bfloat16
    xT = np.ascontiguousarray(x_b.T.astype(np.float32))
    wqkvT = np.ascontiguousarray(w_qkv.T.astype(np.float32))
    wprojT = np.ascontiguousarray(w_proj.T.astype(np.float32))
    # bias_exp[r, i] = -30 * (1 - p[i*128 + r]) per key chunk
    pad = NCH * P - N
    p_pad = np.concatenate([p_b.astype(np.float32), np.zeros(pad, np.float32)])
    bias = (NEG * (1.0 - p_pad)).reshape(NCH, P).T.copy()
    # dfix[:, i, :] = diag(240 * (1 - p_chunk_i)) as bf16
    dfix = np.zeros((P, NCH, P), np.float32)
    for i in range(NCH):
        chunk = p_pad[i * P : (i + 1) * P]
        np.fill_diagonal(dfix[:, i, :], DIAGV * (1.0 - chunk))
    return {
        "xT": xT,
        "wqkvT": wqkvT,
        "wprojT": wprojT,
        "bias_exp": np.ascontiguousarray(bias),
        "dfix": dfix.astype(bf16),
        "ident": np.eye(P, dtype=np.float32).astype(bf16),
        "bvec": b_proj.reshape(1, C).astype(np.float32),
    }


def _install_ntff_hook():
    """The container's antenv package lacks axon_hooks; recreate the NTFF
    profile hook (mirrors trn_agent_boot) so trace=True yields exec_time."""
    import types
    import ctypes
    import contextlib

    if "antenv.axon_hooks" in sys.modules:
        return
    so_path = "/opt/axon/libaxon_pjrt.so"
    mod = types.ModuleType("antenv.axon_hooks")
    state = {"hook": None}
    mod.set_axon_ntff_profile_hook = lambda h: state.__setitem__("hook", h)
    mod.get_axon_ntff_profile_hook = lambda: state["hook"]
    sys.modules["antenv.axon_hooks"] = mod

    try:
        lib = ctypes.CDLL(so_path)
    except OSError:
        return
    if not hasattr(lib, "axon_start_nrt_profile"):
        return
    lib.axon_start_nrt_profile.argtypes = [
        ctypes.POINTER(ctypes.c_int64),
        ctypes.c_size_t,
    ]
    lib.axon_start_nrt_profile.restype = ctypes.c_int64
    lib.axon_stop_nrt_profile.argtypes = [ctypes.c_char_p]
    lib.axon_stop_nrt_profile.restype = ctypes.c_int64

    @contextlib.contextmanager
    def _hook(output_dir, device_ids):
        import jax

        jax.devices()
        if device_ids:
            ids = (ctypes.c_int64 * len(device_ids))(*device_ids)
            rc = lib.axon_start_nrt_profile(ids, len(device_ids))
        else:
            rc = lib.axon_start_nrt_profile(None, 0)
        if rc != 0:
            raise RuntimeError(f"axon_start_nrt_profile rc={rc}")
        try:
            yield
        finally:
            n = lib.axon_stop_nrt_profile(str(output_dir).encode())
            print(f"profile: {n} file(s) written to {output_dir}", file=sys.stderr)

    state["hook"] = _hook


def kernel(x, vis_tube, w_qkv, w_proj, b_proj, _trace=False):
    from concourse.bass_utils import run_bass_kernel_spmd

    if _trace:
        _install_ntff_hook()

    if "nc" not in _CACHE:
        _CACHE["nc"] = _build_nc()
    nc = _CACHE["nc"]

    x = np.asarray(x, np.float32)
    p = np.asarray(vis_tube, np.float32)[:, :, 0]
    in_maps = [
        _prep_core_inputs(x[b], p[b], np.asarray(w_qkv), np.asarray(w_proj),
                          np.asarray(b_proj))
        for b in range(B)
    ]
    res = run_bass_kernel_spmd(nc, in_maps, core_ids=list(range(B)), trace=_trace)
    out = np.stack([res.results[i]["out"] for i in range(B)], axis=0)
    if _trace:
        _CACHE["last_result"] = res
    return out


# revision 18
# speedup vs baseline: 1.1828x; 1.1828x over previous
"""Trainium2 Bass kernel for policy-masked attention (nn_Attention_5007931867377).

Reference computation (per batch b):
    qkv = x @ w_qkv.T ; split into q,k,v heads [H=6, N=1568, D=64]
    s   = (q @ k.T) * D**-0.5
    mask[m] visibility per key + diagonal always kept
    e   = exp(s - max) * mask ; attn = (e + EPS/N)/(sum e + EPS)
    out = (attn @ v) concat heads @ w_proj.T + b_proj

Strategy: pure data parallel, one batch element per NeuronCore (8 cores).
Per-core dataflow (everything transposed so softmax reductions are on the
free axis and e^T feeds the V-matmul without on-chip transposes):
  - host passes x^T (bf16); on chip: Q^T,K^T in [c_out, n] layout, V in [n, d]
  - scores^T[m, q] = K^T.T @ Q^T  (PSUM, per 128-key chunk x query piece)
  - key-mask folded into exp as per-partition bias (-30 * (1-p));
    diagonal-keep folded into scores via +240*(1-p) diag matmul (exp scale
    0.125 makes that +30, cancelling the -30 bias exactly on the diagonal)
  - e^T = exp(0.125*s + bias) ACT -> SBUF (bf16)
  - V augmented with a ones column: outT_aug[65, q] = V_aug.T @ e^T gives
    attention output rows 0..63 and the softmax denominator in row 64
  - normalize with reciprocal_approx_fast + a rank-1 broadcast matmul
  - proj: y[n, :] = oT.T @ w_proj.T (+ bias via K=1 ones matmul), DMA out.

All matmuls in bf16: float32r matmuls don't register as PE activity for
the HAM clock gate (profiled: K=4/8 at 1.2 GHz for 90% of the kernel) and
their 4-byte fused weight loads serialize; bf16 runs warm at 2.4 GHz with
fast weight load. Max-subtraction is dropped: scores ~ N(0,1), |s| < ~6.
"""

import sys

if "/opt/trn_rl_repo" not in sys.path:
    sys.path.insert(0, "/opt/trn_rl_repo")

import numpy as np

B, N, C, H = 8, 1568, 384, 6
D = C // H  # 64
SCALE = D ** -0.5  # 0.125
EPS = 1e-6
NEG = -30.0  # masked-key exp bias (exp(-30) ~ 9e-14, way below EPS/N)
DIAGV = -NEG / SCALE  # 240.0 added to diagonal scores, cancels bias exactly

P = 128
NCH = (N + P - 1) // P  # 13 key/token chunks (12 x 128 + 1 x 32)
CHS = [min(P, N - i * P) for i in range(NCH)]
# query pieces: 128-aligned, <=2 PSUM banks each, subsplit to 512 for matmuls
QPIECES = [(0, 896), (896, 672)]


def _subsplit(qo, qw, step=512):
    return [(qo + o, min(step, qw - o)) for o in range(0, qw, step)]


def _banksplit(qo, qw, step=512):
    """Split [qo, qo+qw) at global multiples of `step` (PSUM bank bounds)."""
    out = []
    o = qo
    while o < qo + qw:
        nxt = min((o // step + 1) * step, qo + qw)
        out.append((o, nxt - o))
        o = nxt
    return out


_CACHE = {}


def _build_nc():
    import concourse.tile as tile
    from concourse import bacc, mybir

    dt = mybir.dt
    f32 = dt.float32
    bf16 = dt.bfloat16
    AF = mybir.ActivationFunctionType

    nc = bacc.Bacc()

    xT_d = nc.declare_dram_parameter("xT", [C, N], bf16, isOutput=False)
    wqkvT_d = nc.declare_dram_parameter("wqkvT", [C, 3 * C], bf16, isOutput=False)
    wprojT_d = nc.declare_dram_parameter("wprojT", [C, C], bf16, isOutput=False)
    bias_d = nc.declare_dram_parameter("bias_exp", [P, NCH], f32, isOutput=False)
    dfix_d = nc.declare_dram_parameter("dfix", [P, NCH, P], bf16, isOutput=False)
    ident_d = nc.declare_dram_parameter("ident", [P, P], bf16, isOutput=False)
    bvec_d = nc.declare_dram_parameter("bvec", [1, C], bf16, isOutput=False)
    out_d = nc.declare_dram_parameter("out", [N, C], f32, isOutput=True)

    with tile.TileContext(nc) as tc:
        with (
            tc.tile_pool(name="persist", bufs=1) as pp,
            tc.tile_pool(name="work", bufs=4) as wp,
        ):
            # ---- persistent SBUF tensors ----
            xt = pp.tile([P, 3, N], bf16, tag="xt")  # x^T chunks (c rows)
            wqkv = pp.tile([P, 3, 3 * C], bf16, tag="wqkv")
            wproj = pp.tile([P, 3, C], bf16, tag="wproj")
            qk = pp.tile([P, 6, N], bf16, tag="qk")  # Q^T (0..2), K^T (3..5)
            vaug = pp.tile([P, NCH, H, D + 1], bf16, tag="vaug")
            ot = pp.tile([P, 3, N], bf16, tag="ot")  # normalized attn out ^T
            bias = pp.tile([P, NCH], f32, tag="bias")
            dfix = pp.tile([P, NCH, P], bf16, tag="dfix")
            ident = pp.tile([P, P], bf16, tag="ident")
            bvec = pp.tile([1, C], bf16, tag="bvec")
            ones = pp.tile([1, P], bf16, tag="ones")

            nc.sync.dma_start(xt[:, :, :], xT_d[:].rearrange("(a p) n -> p a n", p=P))
            nc.sync.dma_start(
                wqkv[:, :, :], wqkvT_d[:].rearrange("(a p) n -> p a n", p=P)
            )
            nc.sync.dma_start(
                wproj[:, :, :], wprojT_d[:].rearrange("(a p) n -> p a n", p=P)
            )
            nc.sync.dma_start(bias[:, :], bias_d[:])
            nc.sync.dma_start(dfix[:, :, :], dfix_d[:])
            nc.sync.dma_start(ident[:, :], ident_d[:])
            nc.sync.dma_start(bvec[:, :], bvec_d[:])
            nc.vector.memset(ones[:, :], 1.0)

            # ---- phase 1: qkv projections ----
            with tc.tile_pool(name="qkvps", bufs=3, space="PSUM") as qps:
                # Q^T / K^T: out[c_out 128, n] = wqkvT_chunk.T @ xT
                for cc in range(6):  # 6 chunks of 128 output channels (q, k)
                    for (qo, qw) in _subsplit(0, N):
                        ps = qps.tile([P, 512], f32, tag="qk")
                        for c in range(3):
                            nc.tensor.matmul(
                                ps[:, :qw],
                                wqkv[:, c, cc * P : (cc + 1) * P],
                                xt[:, c, qo : qo + qw],
                                start=(c == 0),
                                stop=(c == 2),
                            )
                        nc.vector.tensor_copy(qk[:, cc, qo : qo + qw], ps[:, :qw])
                # V: out[n 128, 384] = xT_chunk.T @ w_vT ; pack into vaug
                for i in range(NCH):
                    m = CHS[i]
                    ps = qps.tile([P, C], f32, tag="v")
                    for c in range(3):
                        nc.tensor.matmul(
                            ps[:m, :],
                            xt[:, c, i * P : i * P + m],
                            wqkv[:, c, 2 * C : 3 * C],
                            start=(c == 0),
                            stop=(c == 2),
                        )
                    nc.vector.tensor_copy(
                        vaug[:m, i, :, 0:D],
                        ps[:m, :].rearrange("p (h d) -> p h d", h=H),
                    )
                    nc.vector.memset(vaug[:m, i, :, D : D + 1], 1.0)

            # ---- phase 2: attention per head ----
            # NOTE: matmul start=True clears has_written for the whole PSUM
            # bank, so each accumulation region must own its banks: one
            # bank-aligned outp tile per query piece.
            with (
                tc.tile_pool(name="outps", bufs=2, space="PSUM") as ops,
                tc.tile_pool(name="scps", bufs=2, space="PSUM") as sps,
            ):
                for h in range(H):
                    qc, qr = h // 2, (h % 2) * D  # Q^T chunk / row of head h
                    kc, kr = 3 + h // 2, (h % 2) * D
                    outp0 = ops.tile([D + 1, 896], f32, tag="outT")
                    outp1 = ops.tile([D + 1, 896], f32, tag="outT")
                    outps = [outp0, outp1]
                    for i in range(NCH):
                        m = CHS[i]
                        for pi, (qo, qw) in enumerate(QPIECES):
                            sc = sps.tile([P, 896], f32, tag="sc")
                            # diag block of chunk i lives at cols i*P..i*P+m
                            dlo, dhi = i * P, i * P + m
                            has_diag = qo <= dlo < qo + qw
                            subs = _subsplit(qo, qw)
                            for si, (so, sw) in enumerate(subs):
                                nc.tensor.matmul(
                                    sc[:m, so - qo : so - qo + sw],
                                    qk[kr : kr + D, kc, i * P : i * P + m],
                                    qk[qr : qr + D, qc, so : so + sw],
                                    start=True,
                                    stop=(not has_diag) and si == len(subs) - 1,
                                )
                            if has_diag:
                                nc.tensor.matmul(
                                    sc[:m, dlo - qo : dhi - qo],
                                    ident[:, :m],
                                    dfix[:, i, :m],
                                    start=False,
                                    stop=True,
                                )
                            et = wp.tile([P, 896], bf16, tag="et")
                            nc.scalar.activation(
                                et[:m, :qw],
                                sc[:m, :qw],
                                AF.Exp,
                                bias=bias[:m, i : i + 1],
                                scale=SCALE,
                            )
                            for (so, sw) in subs:
                                nc.tensor.matmul(
                                    outps[pi][:, so - qo : so - qo + sw],
                                    vaug[:m, i, h, :],
                                    et[:m, so - qo : so - qo + sw],
                                    start=(i == 0),
                                    stop=(i == NCH - 1),
                                )
                    # normalization: S = row 64 of outp pieces
                    srow = wp.tile([1, N], f32, tag="srow")
                    srow_b = wp.tile([1, N], bf16, tag="srow_b")
                    for pi, (qo, qw) in enumerate(QPIECES):
                        nc.vector.tensor_scalar_add(
                            srow[:, qo : qo + qw],
                            outps[pi][D : D + 1, :qw],
                            EPS,
                        )
                    nc.vector.reciprocal_approx_fast(srow[:, :], srow[:, :])
                    nc.vector.tensor_copy(srow_b[:, :], srow[:, :])
                    for pi, (qo, qw) in enumerate(QPIECES):
                        rb = sps.tile([P, 896], f32, tag="sc")  # reuse bank slots
                        for (so, sw) in _subsplit(qo, qw):
                            nc.tensor.matmul(
                                rb[:D, so - qo : so - qo + sw],
                                ones[:, :D],
                                srow_b[:, so : so + sw],
                            )
                        rbs = wp.tile([D, 896], f32, tag="rbs")
                        nc.vector.tensor_copy(rbs[:, :qw], rb[:D, :qw])
                        nc.vector.tensor_mul(
                            ot[qr : qr + D, qc, qo : qo + qw],
                            outps[pi][0:D, :qw],
                            rbs[:, :qw],
                        )

            # ---- phase 3: output projection ----
            with tc.tile_pool(name="yps", bufs=3, space="PSUM") as yps:
                for j in range(NCH):
                    m = CHS[j]
                    yp = yps.tile([P, C], f32, tag="y")
                    for c in range(3):
                        nc.tensor.matmul(
                            yp[:m, :],
                            ot[:, c, j * P : j * P + m],
                            wproj[:, c, :],
                            start=(c == 0),
                            stop=False,
                        )
                    nc.tensor.matmul(
                        yp[:m, :],
                        ones[:, :m],
                        bvec[:, :],
                        start=False,
                        stop=True,
                    )
                    ys = wp.tile([P, C], f32, tag="ys")
                    nc.vector.tensor_copy(ys[:m, :], yp[:m, :])
                    nc.sync.dma_start(out_d[j * P : j * P + m, :], ys[:m, :])

    nc.finalize()
    return nc


def _prep_core_inputs(x_b, p_b, w_qkv, w_proj, b_proj):
    import ml_dtypes

    bf16 = ml_dtypes.bfloat16
    xT = np.ascontiguousarray(x_b.T).astype(bf16)
    wqkvT = np.ascontiguousarray(w_qkv.T).astype(bf16)
    wprojT = np.ascontiguousarray(w_proj.T).astype(bf16)
    # bias_exp[r, i] = -30 * (1 - p[i*128 + r]) per key chunk
    pad = NCH * P - N
    p_pad = np.concatenate([p_b.astype(np.float32), np.zeros(pad, np.float32)])
    bias = (NEG * (1.0 - p_pad)).reshape(NCH, P).T.copy()
    # dfix[:, i, :] = diag(240 * (1 - p_chunk_i)) as bf16
    dfix = np.zeros((P, NCH, P), np.float32)
    for i in range(NCH):
        chunk = p_pad[i * P : (i + 1) * P]
        np.fill_diagonal(dfix[:, i, :], DIAGV * (1.0 - chunk))
    return {
        "xT": xT,
        "wqkvT": wqkvT,
        "wprojT": wprojT,
        "bias_exp": np.ascontiguousarray(bias),
        "dfix": dfix.astype(bf16),
        "ident": np.eye(P, dtype=np.float32).astype(bf16),
        "bvec": b_proj.reshape(1, C).astype(np.float32).astype(bf16),
    }


def _install_ntff_hook():
    """The container's antenv package lacks axon_hooks; recreate the NTFF
    profile hook (mirrors trn_agent_boot) so trace=True yields exec_time."""
    import types
    import ctypes
    import contextlib

    if "antenv.axon_hooks" in sys.modules:
        return
    so_path = "/opt/axon/libaxon_pjrt.so"
    mod = types.ModuleType("antenv.axon_hooks")
    state = {"hook": None}
    mod.set_axon_ntff_profile_hook = lambda h: state.__setitem__("hook", h)
    mod.get_axon_ntff_profile_hook = lambda: state["hook"]
    sys.modules["antenv.axon_hooks"] = mod

    try:
        lib = ctypes.CDLL(so_path)
    except OSError:
        return
    if not hasattr(lib, "axon_start_nrt_profile"):
        return
    lib.axon_start_nrt_profile.argtypes = [
        ctypes.POINTER(ctypes.c_int64),
        ctypes.c_size_t,
    ]
    lib.axon_start_nrt_profile.restype = ctypes.c_int64
    lib.axon_stop_nrt_profile.argtypes = [ctypes.c_char_p]
    lib.axon_stop_nrt_profile.restype = ctypes.c_int64

    @contextlib.contextmanager
    def _hook(output_dir, device_ids):
        import jax

        jax.devices()
        if device_ids:
            ids = (ctypes.c_int64 * len(device_ids))(*device_ids)
            rc = lib.axon_start_nrt_profile(ids, len(device_ids))
        else:
            rc = lib.axon_start_nrt_profile(None, 0)
        if rc != 0:
            raise RuntimeError(f"axon_start_nrt_profile rc={rc}")
        try:
            yield
        finally:
            n = lib.axon_stop_nrt_profile(str(output_dir).encode())
            print(f"profile: {n} file(s) written to {output_dir}", file=sys.stderr)

    state["hook"] = _hook


def kernel(x, vis_tube, w_qkv, w_proj, b_proj, _trace=False):
    from concourse.bass_utils import run_bass_kernel_spmd

    if _trace:
        _install_ntff_hook()

    if "nc" not in _CACHE:
        _CACHE["nc"] = _build_nc()
    nc = _CACHE["nc"]

    x = np.asarray(x, np.float32)
    p = np.asarray(vis_tube, np.float32)[:, :, 0]
    in_maps = [
        _prep_core_inputs(x[b], p[b], np.asarray(w_qkv), np.asarray(w_proj),
                          np.asarray(b_proj))
        for b in range(B)
    ]
    res = run_bass_kernel_spmd(nc, in_maps, core_ids=list(range(B)), trace=_trace)
    out = np.stack([res.results[i]["out"] for i in range(B)], axis=0)
    if _trace:
        _CACHE["last_result"] = res
    return out


# revision 60
# speedup vs baseline: 2.4173x; 2.0438x over previous
"""Trainium2 Bass kernel for policy-masked attention (nn_Attention_5007931867377).

Reference computation (per batch b):
    qkv = x @ w_qkv.T ; split into q,k,v heads [H=6, N=1568, D=64]
    s   = (q @ k.T) * D**-0.5
    mask[m] visibility per key + diagonal always kept
    e   = exp(s - max) * mask ; attn = (e + EPS/N)/(sum e + EPS)
    out = (attn @ v) concat heads @ w_proj.T + b_proj

Strategy: pure data parallel, one batch element per NeuronCore (8 cores).
Per-core dataflow (everything transposed so softmax reductions are on the
free axis and e^T feeds the V-matmul without on-chip transposes):
  - host passes x^T (bf16); on chip: Q^T,K^T in [c_out, n] layout, V in [n, d]
  - scores^T[m, q] = K^T.T @ Q^T  (PSUM, per 128-key chunk x query piece)
  - key-mask folded into exp as per-partition bias (-30 * (1-p));
    diagonal-keep folded into scores via +240*(1-p) diag matmul (exp scale
    0.125 makes that +30, cancelling the -30 bias exactly on the diagonal)
  - e^T = exp(0.125*s + bias) ACT -> SBUF (bf16)
  - V augmented with a ones column: outT_aug[65, q] = V_aug.T @ e^T gives
    attention output rows 0..63 and the softmax denominator in row 64
  - normalize with reciprocal_approx_fast + a rank-1 broadcast matmul
  - proj: y[n, :] = oT.T @ w_proj.T (+ bias via K=1 ones matmul), DMA out.

All matmuls in bf16: float32r matmuls don't register as PE activity for
the HAM clock gate (profiled: K=4/8 at 1.2 GHz for 90% of the kernel) and
their 4-byte fused weight loads serialize; bf16 runs warm at 2.4 GHz with
fast weight load. Max-subtraction is dropped: scores ~ N(0,1), |s| < ~6.
"""

import sys

if "/opt/trn_rl_repo" not in sys.path:
    sys.path.insert(0, "/opt/trn_rl_repo")

import numpy as np

B, N, C, H = 8, 1568, 384, 6
D = C // H  # 64
SCALE = D ** -0.5  # 0.125
EPS = 1e-6
NEG = -30.0  # masked-key exp bias (exp(-30) ~ 9e-14, way below EPS/N)
DIAGV = -NEG / SCALE  # 240.0 added to diagonal scores, cancels bias exactly

P = 128
NCH = (N + P - 1) // P  # 13 key/token chunks (12 x 128 + 1 x 32)
CHS = [min(P, N - i * P) for i in range(NCH)]
# query pieces: 128-aligned, <=2 PSUM banks each, subsplit to 512 for matmuls
QPIECES = [(0, 512), (512, 512), (1024, 512), (1536, 32)]
SLOTW = 512
NPIECE = len(QPIECES)


def _subsplit(qo, qw, step=512):
    return [(qo + o, min(step, qw - o)) for o in range(0, qw, step)]


def _banksplit(qo, qw, step=512):
    """Split [qo, qo+qw) at global multiples of `step` (PSUM bank bounds)."""
    out = []
    o = qo
    while o < qo + qw:
        nxt = min((o // step + 1) * step, qo + qw)
        out.append((o, nxt - o))
        o = nxt
    return out


_CACHE = {}
UPFRONT_QKV = False


def _build_nc(KC, BSTART, HASB):
    """Build the SPMD program.

    Tokens are permuted host-side so policy-kept keys come first. KC = number
    of 128-key chunks holding any kept key (full scores+softmax+V path);
    chunks BSTART..NCH-1 contain dropped keys, which are visible only to
    their own query (diagonal) — handled by tiny 128x128 diagonal-block
    tasks (constant -30 exp bias + dfix diag matmul restores exactly the
    (1-p) diagonal entries).
    """
    import concourse.tile as tile
    from concourse import bacc, mybir

    dt = mybir.dt
    f32 = dt.float32
    bf16 = dt.bfloat16
    AF = mybir.ActivationFunctionType

    nc = bacc.Bacc()

    xT_d = nc.declare_dram_parameter("xT", [C, N], bf16, isOutput=False)
    wqkvT_d = nc.declare_dram_parameter("wqkvT", [C, 3 * C], bf16, isOutput=False)
    wprojT_d = nc.declare_dram_parameter("wprojT", [C, C], bf16, isOutput=False)
    bias_d = nc.declare_dram_parameter("bias_exp", [P, NCH], f32, isOutput=False)
    dfix_d = nc.declare_dram_parameter("dfix", [P, NCH, P], bf16, isOutput=False)
    ident_d = nc.declare_dram_parameter("ident", [P, P], bf16, isOutput=False)
    bvec_d = nc.declare_dram_parameter("bvec", [1, C], bf16, isOutput=False)
    out_d = nc.declare_dram_parameter("out", [N, C], f32, isOutput=True)

    with tile.TileContext(nc, pool_alloc_mode="queue") as tc:
        with (
            tc.tile_pool(name="persist", bufs=1) as pp,
            tc.tile_pool(name="work", bufs=6) as wp,
        ):
            # ---- persistent SBUF tensors ----
            xt = pp.tile([P, 3, N], bf16, tag="xt")  # x^T chunks (c rows)
            wqkv = pp.tile([P, 3, 3 * C], bf16, tag="wqkv")
            wproj = pp.tile([P, 3, C], bf16, tag="wproj")
            qk = pp.tile([P, 6, N], bf16, tag="qk")  # Q^T (0..2), K^T (3..5)
            vaug = pp.tile([P, NCH, H, D + 1], bf16, tag="vaug")
            ot = pp.tile([P, 3, N], bf16, tag="ot")  # normalized attn out ^T
            bias = pp.tile([P, NCH], f32, tag="bias")
            dfix = pp.tile([P, NCH, P], bf16, tag="dfix")
            ident = pp.tile([P, P], bf16, tag="ident")
            bvec = pp.tile([1, C], bf16, tag="bvec")
            ones = pp.tile([1, P], bf16, tag="ones")
            negb = pp.tile([P, 1], f32, tag="negb")  # band exp bias (-30)

            # split big input DMAs per c-chunk so they round-robin across DMA
            # queues and the first qkv matmul starts as early as possible
            xr = xT_d[:].rearrange("(a p) n -> p a n", p=P)
            qr_ = wqkvT_d[:].rearrange("(a p) n -> p a n", p=P)
            pr = wprojT_d[:].rearrange("(a p) n -> p a n", p=P)
            for c in range(3):
                nc.sync.dma_start(xt[:, c, :], xr[:, c, :])
                nc.sync.dma_start(wqkv[:, c, :], qr_[:, c, :])
            for c in range(3):
                nc.sync.dma_start(wproj[:, c, :], pr[:, c, :])
            nc.sync.dma_start(bias[:, :], bias_d[:])
            nc.sync.dma_start(dfix[:, :, :], dfix_d[:])
            nc.sync.dma_start(ident[:, :], ident_d[:])
            nc.sync.dma_start(bvec[:, :], bvec_d[:])
            nc.vector.memset(ones[:, :], 1.0)
            nc.vector.memset(negb[:, :], NEG)
            # dummy exp so the ACT table set loads during the DMA prologue
            warm = pp.tile([1, 1], f32, tag="warm")
            nc.scalar.activation(warm[:, :], negb[0:1, :], AF.Exp)

            # ---- phase 1: qkv projections ----
            # Only the head-pair-0 channels (cc 0 and 3) and V are computed
            # up front; cc 1/4 (pair 1) and cc 2/5 (pair 2) are interleaved
            # into the attention task stream of the previous pair, giving the
            # PE independent work while it would otherwise wait on exp.
            def _qkv_unit(pool, tag, cc, qo, qw):
                def emit():
                    ps = pool.tile([P, 512], f32, tag=tag,
                                   name=f"qp{cc}_{qo}")
                    for c in range(3):
                        nc.tensor.matmul(
                            ps[:, :qw],
                            wqkv[:, c, cc * P : (cc + 1) * P],
                            xt[:, c, qo : qo + qw],
                            start=(c == 0),
                            stop=(c == 2),
                        )
                    nc.vector.tensor_copy(qk[:, cc, qo : qo + qw], ps[:, :qw])
                return emit

            with tc.tile_pool(name="qkvps", bufs=3, space="PSUM") as qps:
                for cc in ((0, 1, 2, 3, 4, 5) if UPFRONT_QKV else (0, 3)):
                    for (qo, qw) in _subsplit(0, N):
                        _qkv_unit(qps, "qk", cc, qo, qw)()
                # V: out[n 128, 384] = xT_chunk.T @ w_vT ; pack into vaug
                for i in range(NCH):
                    m = CHS[i]
                    ps = qps.tile([P, C], f32, tag="v")
                    for c in range(3):
                        nc.tensor.matmul(
                            ps[:m, :],
                            xt[:, c, i * P : i * P + m],
                            wqkv[:, c, 2 * C : 3 * C],
                            start=(c == 0),
                            stop=(c == 2),
                        )
                    nc.vector.tensor_copy(
                        vaug[:m, i, :, 0:D],
                        ps[:m, :].rearrange("p (h d) -> p h d", h=H),
                    )
                    nc.vector.memset(vaug[:m, i, :, D : D + 1], 1.0)

            # ---- phase 2: attention, software-pipelined across (head, chunk)
            # NOTES:
            #  - matmul start=True clears has_written for the whole PSUM bank,
            #    so each accumulation region owns its banks (one outp tile per
            #    query piece).
            #  - the HAM clock gate re-throttles PE to 1.2 GHz unless the MM
            #    stream is dense; scores of task t+1 are emitted before the
            #    exp-gated V-matmuls of task t so PE always has ready work.
            with (
                tc.tile_pool(name="outps", bufs=4, space="PSUM") as ops,
                tc.tile_pool(name="scps", bufs=4, space="PSUM") as sps,
            ):
                outps = {}  # head -> [piece tiles]
                scs = {}  # task -> [sc tiles per piece] / [sc tile] for band
                ets = {}  # task -> matching et tiles

                # Which vmm is the last writer of each (piece, sub-region)?
                # Emission interleaves kept k_i (position 2i) and band b_j
                # (position 2(j-BSTART)+1): band j comes after kept KC-1 only
                # when j-BSTART >= KC-1.
                last_band = {}  # region -> band j that is the final writer
                kept_is_last = {}  # region -> kept KC-1 is the final writer
                for pi, (qo, qw) in enumerate(QPIECES):
                    for (so, sw) in _subsplit(qo, qw):
                        js = [j for j in range(BSTART, NCH)
                              if so <= j * P and j * P + CHS[j] <= so + sw]
                        lb = max(js) if js else None
                        if lb is not None and lb - BSTART >= KC - 1:
                            last_band[(pi, so)] = lb
                            kept_is_last[(pi, so)] = False
                        else:
                            last_band[(pi, so)] = None
                            kept_is_last[(pi, so)] = True

                def _piece_of(j):
                    for pi, (qo, qw) in enumerate(QPIECES):
                        if qo <= j * P < qo + qw:
                            for (so, sw) in _subsplit(qo, qw):
                                if so <= j * P and j * P + CHS[j] <= so + sw:
                                    return pi, qo, so
                    raise AssertionError

                def emit_scores(task, pieces=tuple(range(NPIECE))):
                    h, i, kind = task
                    kc, kr = 3 + h // 2, (h % 2) * D
                    qc, qr = h // 2, (h % 2) * D
                    m = CHS[i]
                    if kind == "band":
                        # diagonal-only block: keys chunk i vs queries chunk i
                        sc = sps.tile([P, SLOTW], f32, tag="sc",
                                      name=f"sb{h}_{i}")
                        nc.tensor.matmul(
                            sc[:m, :m],
                            qk[kr : kr + D, kc, i * P : i * P + m],
                            qk[qr : qr + D, qc, i * P : i * P + m],
                            start=True,
                            stop=False,
                        )
                        nc.tensor.matmul(
                            sc[:m, :m],
                            ident[:, :m],
                            dfix[:, i, :m],
                            start=False,
                            stop=True,
                        )
                        scs[task] = sc
                        return
                    if task not in scs:
                        scs[task] = [None] * len(QPIECES)
                    for pi in pieces:
                        qo, qw = QPIECES[pi]
                        sc = sps.tile([P, SLOTW], f32, tag="sc", name=f"sc{h}_{i}")
                        subs = _subsplit(qo, qw)
                        for si, (so, sw) in enumerate(subs):
                            nc.tensor.matmul(
                                sc[:m, so - qo : so - qo + sw],
                                qk[kr : kr + D, kc, i * P : i * P + m],
                                qk[qr : qr + D, qc, so : so + sw],
                                start=True,
                                stop=si == len(subs) - 1,
                            )
                        scs[task][pi] = sc

                def emit_exp(task):
                    h, i, kind = task
                    m = CHS[i]
                    if kind == "band":
                        et = wp.tile([P, SLOTW], bf16, tag="et", name=f"eb{h}_{i}")
                        nc.scalar.activation(
                            et[:m, :m],
                            scs[task][:m, :m],
                            AF.Exp,
                            bias=negb[:m, :],
                            scale=SCALE,
                        )
                        ets[task] = et
                        return
                    tiles = []
                    for pi, (qo, qw) in enumerate(QPIECES):
                        et = wp.tile([P, SLOTW], bf16, tag="et", name=f"et{h}_{i}")
                        nc.scalar.activation(
                            et[:m, :qw],
                            scs[task][pi][:m, :qw],
                            AF.Exp,
                            bias=bias[:m, i : i + 1],
                            scale=SCALE,
                        )
                        tiles.append(et)
                    ets[task] = tiles

                def emit_vmm(task, pieces=tuple(range(NPIECE))):
                    h, i, kind = task
                    m = CHS[i]
                    if kind == "band":
                        pi, qo, so = _piece_of(i)
                        off = i * P - qo
                        nc.tensor.matmul(
                            outps[h][pi][:, off : off + m],
                            vaug[:m, i, h, :],
                            ets[task][:m, :m],
                            start=False,
                            stop=last_band[(pi, so)] == i,
                        )
                        del ets[task], scs[task]
                        return
                    if i == 0 and h not in outps:
                        outps[h] = []
                        for _pi in range(NPIECE):
                            o_ = ops.tile([D + 1, SLOTW], f32, tag="outT",
                                          name=f"o{_pi}_{h}")
                            outps[h].append(o_)
                    for pi in pieces:
                        qo, qw = QPIECES[pi]
                        for (so, sw) in _subsplit(qo, qw):
                            nc.tensor.matmul(
                                outps[h][pi][:, so - qo : so - qo + sw],
                                vaug[:m, i, h, :],
                                ets[task][pi][:m, so - qo : so - qo + sw],
                                start=(i == 0),
                                stop=(i == KC - 1 and kept_is_last[(pi, so)]),
                            )
                    if pieces[-1] == len(QPIECES) - 1:
                        del ets[task], scs[task]

                def emit_proj(j):
                    # output projection chunk j (reuses sc PSUM slots)
                    m = CHS[j]
                    yp = sps.tile([P, SLOTW], f32, tag="sc", name=f"yp{j}")
                    for c in range(3):
                        nc.tensor.matmul(
                            yp[:m, :C],
                            ot[:, c, j * P : j * P + m],
                            wproj[:, c, :],
                            start=(c == 0),
                            stop=(c == 2 and not HASB),
                        )
                    if HASB:
                        nc.tensor.matmul(
                            yp[:m, :C],
                            ones[:, :m],
                            bvec[:, :],
                            start=False,
                            stop=True,
                        )
                    ys = wp.tile([P, C], f32, tag="ys", name=f"ys{j}")
                    nc.vector.tensor_copy(ys[:m, :], yp[:m, :C])
                    nc.sync.dma_start(out_d[j * P : j * P + m, :], ys[:m, :])

                def emit_norm(h, pi):
                    # normalization via gpsimd partition_broadcast (PE-free):
                    # copy S row, broadcast to 64 partitions, reciprocal, mul.
                    # +EPS dropped: S >= the always-kept diagonal term
                    # (>~0.1), so eps=1e-6 is noise.
                    qc, qr = h // 2, (h % 2) * D
                    qo, qw = QPIECES[pi]
                    srow = wp.tile([1, SLOTW], f32, tag="srow", name=f"sr{h}{pi}")
                    nc.vector.tensor_copy(
                        srow[:, :qw], outps[h][pi][D : D + 1, :qw]
                    )
                    rbr = wp.tile([D, SLOTW], f32, tag="rbr", name=f"rr{h}{pi}")
                    nc.gpsimd.partition_broadcast(rbr[:, :qw], srow[:, :qw])
                    rbs = wp.tile([D, SLOTW], f32, tag="rbs", name=f"rs{h}{pi}")
                    nc.vector.reciprocal_approx_fast(rbs[:, :qw], rbr[:, :qw])
                    nc.vector.tensor_mul(
                        ot[qr : qr + D, qc, qo : qo + qw],
                        outps[h][pi][0:D, :qw],
                        rbs[:, :qw],
                    )

                # interleave band tasks between kept tasks: band scores are
                # independent PE work that fills exp-wait gaps
                tasks = []
                norm_after = {}  # task index -> [(h, piece)] to normalize
                for h in range(H):
                    ht = [(h, i, "kept") for i in range(KC)]
                    bt = [(h, j, "band") for j in range(BSTART, NCH)]
                    merged = []
                    while ht or bt:
                        if ht:
                            merged.append(ht.pop(0))
                        if bt:
                            merged.append(bt.pop(0))
                    base = len(tasks)
                    for pi in range(len(QPIECES)):
                        last = None
                        for k, (_, i, kind) in enumerate(merged):
                            if kind == "kept" or _piece_of(i)[0] == pi:
                                last = k
                        norm_after.setdefault(base + last, []).append((h, pi))
                    tasks += merged

                # cc 1/4 interleave into head-pair-0 tasks, cc 2/5 into
                # pair 1 (their consumers are pairs 1 and 2 respectively)
                deferred = {}
                if not UPFRONT_QKV:
                    for wave, ccs in enumerate(((1, 4), (2, 5))):
                        units = [_qkv_unit(sps, "sc", cc, qo, qw)
                                 for cc in ccs for (qo, qw) in _subsplit(0, N)]
                        start = sum(1 for t in tasks if t[0] < 2 * wave)
                        for u, unit in enumerate(units):
                            deferred.setdefault(start + 2 * u, []).append(unit)

                emit_scores(tasks[0])
                for t, task in enumerate(tasks):
                    emit_exp(task)
                    nxt = tasks[t + 1] if t + 1 < len(tasks) else None
                    if (nxt is not None and nxt[2] == "kept"
                            and task[2] == "kept"):
                        # fine-grained PE interleave: next scores and current
                        # V-matmuls alternate per query piece
                        for _pi in range(NPIECE):
                            emit_scores(nxt, pieces=(_pi,))
                            emit_vmm(task, pieces=(_pi,))
                    else:
                        if nxt is not None:
                            emit_scores(nxt)
                        emit_vmm(task)
                    for unit in deferred.get(t, []):
                        unit()
                    for (h, pi) in norm_after.get(t, []):
                        emit_norm(h, pi)
                        if h == H - 1:
                            qo, qw = QPIECES[pi]
                            for j in range(NCH):
                                if qo <= j * P and j * P + CHS[j] <= qo + qw:
                                    emit_proj(j)

    nc.finalize()
    return nc


def _prep_core_inputs(x_b, p_b, wqkvT, wprojT, bvec, ident):
    """Permute tokens kept-keys-first; build exp-bias and diag-fix tensors.
    Returns (in_map, perm)."""
    import ml_dtypes

    bf16 = ml_dtypes.bfloat16
    perm = np.argsort(-p_b, kind="stable")
    xT = np.ascontiguousarray(x_b[perm].T).astype(bf16)
    p_perm = p_b[perm].astype(np.float32)
    pad = NCH * P - N
    p_pad = np.concatenate([p_perm, np.zeros(pad, np.float32)])
    # bias_exp[r, i] = -30 * (1 - p[i*128 + r]) per key chunk
    bias = (NEG * (1.0 - p_pad)).reshape(NCH, P).T.copy()
    # dfix[:, i, :] = diag(240 * (1 - p_chunk_i)) as bf16
    dfix = np.zeros((P, NCH, P), np.float32)
    for i in range(NCH):
        chunk = p_pad[i * P : (i + 1) * P]
        np.fill_diagonal(dfix[:, i, :], DIAGV * (1.0 - chunk))
    return {
        "xT": xT,
        "wqkvT": wqkvT,
        "wprojT": wprojT,
        "bias_exp": np.ascontiguousarray(bias),
        "dfix": dfix.astype(bf16),
        "ident": ident,
        "bvec": bvec,
    }, perm


def _install_ntff_hook():
    """The container's antenv package lacks axon_hooks; recreate the NTFF
    profile hook (mirrors trn_agent_boot) so trace=True yields exec_time."""
    import types
    import ctypes
    import contextlib

    if "antenv.axon_hooks" in sys.modules:
        return
    so_path = "/opt/axon/libaxon_pjrt.so"
    mod = types.ModuleType("antenv.axon_hooks")
    state = {"hook": None}
    mod.set_axon_ntff_profile_hook = lambda h: state.__setitem__("hook", h)
    mod.get_axon_ntff_profile_hook = lambda: state["hook"]
    sys.modules["antenv.axon_hooks"] = mod

    try:
        lib = ctypes.CDLL(so_path)
    except OSError:
        return
    if not hasattr(lib, "axon_start_nrt_profile"):
        return
    lib.axon_start_nrt_profile.argtypes = [
        ctypes.POINTER(ctypes.c_int64),
        ctypes.c_size_t,
    ]
    lib.axon_start_nrt_profile.restype = ctypes.c_int64
    lib.axon_stop_nrt_profile.argtypes = [ctypes.c_char_p]
    lib.axon_stop_nrt_profile.restype = ctypes.c_int64

    @contextlib.contextmanager
    def _hook(output_dir, device_ids):
        import jax

        jax.devices()
        if device_ids:
            ids = (ctypes.c_int64 * len(device_ids))(*device_ids)
            rc = lib.axon_start_nrt_profile(ids, len(device_ids))
        else:
            rc = lib.axon_start_nrt_profile(None, 0)
        if rc != 0:
            raise RuntimeError(f"axon_start_nrt_profile rc={rc}")
        try:
            yield
        finally:
            n = lib.axon_stop_nrt_profile(str(output_dir).encode())
            print(f"profile: {n} file(s) written to {output_dir}", file=sys.stderr)

    state["hook"] = _hook


def kernel(x, vis_tube, w_qkv, w_proj, b_proj, _trace=False):
    from concourse.bass_utils import run_bass_kernel_spmd

    import ml_dtypes

    if _trace:
        _install_ntff_hook()

    bf16 = ml_dtypes.bfloat16
    x = np.asarray(x, np.float32)
    p = np.asarray(vis_tube, np.float32)[:, :, 0]
    keeps = (p > 0.5).sum(axis=1)  # kept keys per batch
    KC = max(1, int(-(-keeps.max() // P)))  # chunks containing kept keys
    BSTART = int(keeps.min() // P)  # first chunk containing a dropped key

    HASB = bool(np.any(np.asarray(b_proj)))
    key = (KC, BSTART, HASB)
    if _CACHE.get("key") != key:
        _CACHE["nc"] = _build_nc(KC, BSTART, HASB)
        _CACHE["key"] = key
    nc = _CACHE["nc"]

    wqkvT = np.ascontiguousarray(np.asarray(w_qkv).T).astype(bf16)
    wprojT = np.ascontiguousarray(np.asarray(w_proj).T).astype(bf16)
    bvec = np.asarray(b_proj).reshape(1, C).astype(np.float32).astype(bf16)
    ident = np.eye(P, dtype=np.float32).astype(bf16)
    in_maps, perms = [], []
    for b in range(B):
        im, perm = _prep_core_inputs(x[b], p[b], wqkvT, wprojT, bvec, ident)
        in_maps.append(im)
        perms.append(perm)
    res = run_bass_kernel_spmd(nc, in_maps, core_ids=list(range(B)), trace=_trace)
    out = np.empty((B, N, C), np.float32)
    for b in range(B):
        out[b][perms[b]] = res.results[b]["out"]
    if _trace:
        _CACHE["last_result"] = res
    return out
